# revision 14
# baseline (speedup 1.0000x reference)
"""Mixtral sliding-window attention (B=1, T=2048, C=4096, 32 q heads / 8 kv
heads, D=128, window=1024) on 8 TRN2 NeuronCores.

Sharding: tensor-parallel over kv heads — core c owns kv head c and q heads
4c..4c+3.  Each core computes its q/k/v projections, RoPE, sliding-window
attention, and a partial o_proj (its 512 columns of Wo's input dim); the 8
partial (2048, 4096) outputs are summed on the host.

v1 layout strategy:
  - All operands host-pre-tiled into big per-partition-contiguous blocks so
    every HBM load is one large DMA (~35 DMAs total vs ~400): merged weight /
    x / mask loads, one y store per 128-row block.
  - x is bf16 (mixed-dtype matmuls with f32r weights are allowed); q/k path
    stays f32r for precision; v / wo / probs (ex) / attnT are bf16.
  - RoPE: weights row-permuted (even dims -> partitions 0-63, odd -> 64-127);
    per head 3 DVE muls/adds + 2 ACT half copies using a sign-folded sin
    operand [s; -s].
  - v projection emitted directly in natural [t, d] layout (x-tile stationary,
    wv moving) - no PE transposes, no extra copies.
  - scores computed transposed [tk, tq]; softmax denominators via ones-matmul
    accumulated across k tiles; per-tile q-column ranges trimmed to the
    sliding-window support (exact for bf16 ex streams, min-256 for f32r
    score streams).
  - o_proj interleaved after each tq block's attention so PE never waits on
    the softmax DVE tail.
"""
import math
import os
import sys

sys.path.insert(0, "/opt/trn_rl_repo")
import numpy as np

T = 2048
C = 4096
D = 128
NCORE = 8
HPC = 4          # q heads per core
MQ = HPC * D     # 512 q out dims per core
TQ = 512         # tq block
NTQ = T // TQ    # 4
NCT = C // 128   # 32 contraction tiles
WINDOW = 1024
MASK_ES = [0, 128, 256, 384, -640, -768, -896, -1024]
MASK_IDX = {e: i for i, e in enumerate(MASK_ES)}

LAST_EXEC_NS = None
LAST_RESULTS = None


def _k0_list(q0):
    k0_min = max(0, ((q0 - (WINDOW - 1)) // 128) * 128)
    k0_max = ((q0 + TQ - 1) // 128) * 128
    return list(range(k0_min, k0_max + 1, 128))


def _trim(q0, k0):
    """Valid q-column range [a, b) (relative to q0) for k tile [k0, k0+128)."""
    a = max(0, k0 - q0)
    b = min(TQ, k0 + 127 + WINDOW - q0 + 1)
    return a, b


def _mask_span(q0, k0):
    """Column span (relative to q0) needing the partial-validity mask, or
    None.  e >= 0 (causal diagonal): triangle lives in cols [e, e+128).
    e < 0 (window edge): partial cols are (e+1023, e+1151)."""
    e = k0 - q0
    if e >= 0:
        return e, min(TQ, e + 128)
    lo = e + WINDOW  # first col where the window cuts into this tile
    if lo >= TQ:
        return None
    return max(0, lo), min(TQ, e + 127 + WINDOW + 1)


def _build():
    from concourse import bacc, mybir, tile

    F32 = mybir.dt.float32
    F32R = mybir.dt.float32r
    F16 = mybir.dt.float16
    BF16 = mybir.dt.bfloat16
    AF = mybir.ActivationFunctionType

    nc = bacc.Bacc("TRN2", target_bir_lowering=False, debug=False)

    # host-pre-tiled inputs (all per-partition contiguous)
    xq_d = nc.dram_tensor("xq", (2 * NTQ, 128, 16 * TQ), F16,
                          kind="ExternalInput")      # [tq*2+half][p][cl*512+j]
    wq_d = nc.dram_tensor("wq", (128, NCT * TQ), F16,
                          kind="ExternalInput")      # [p][ct*512 + m]
    wk_d = nc.dram_tensor("wk", (128, NCT * D), F16,
                          kind="ExternalInput")      # [p][ct*128 + d]
    wv_d = nc.dram_tensor("wv", (128, NCT * D), F16,
                          kind="ExternalInput")
    wo_d = nc.dram_tensor("wo", (128, HPC * 8 * TQ), BF16,
                          kind="ExternalInput")      # [p][(m*8+j)*512 + col]
    cs_d = nc.dram_tensor("cs", (128, 2 * T), F32,
                          kind="ExternalInput")      # [[c;c], [s;-s]]
    msk_d = nc.dram_tensor("masks", (128, len(MASK_ES) * TQ), BF16,
                           kind="ExternalInput")
    oneb_d = nc.dram_tensor("onesb", (128, 128), BF16, kind="ExternalInput")
    y_d = nc.dram_tensor("y", (T, C), BF16, kind="ExternalOutput")

    with tile.TileContext(nc) as tc:
        with tc.tile_pool(name="persist", bufs=1) as pp:
            oneb_sb = pp.tile([128, 128], BF16, name="oneb", tag="oneb")

            kTr = [pp.tile([128, TQ], F16, name=f"kTr{i}", tag=f"kTr{i}")
                   for i in range(NTQ)]
            # v_sb[tq] holds the tq block's v in natural [t, d] layout as
            # [128, 4*128]: slice [:, tl*128:+128] is t tile tq*4+tl.
            v_sb = [pp.tile([128, 4 * D], BF16, name=f"v{i}", tag=f"v{i}")
                    for i in range(NTQ)]
            qt_sb = {}   # (h, tq) -> f32r [128, TQ] roped q, transposed [d, t]
            at_sb = {}   # (h, tq) -> bf16 [128, TQ] attnT [d, t]
            for h in range(HPC):
                for tq in range(NTQ):
                    qt_sb[(h, tq)] = pp.tile([128, TQ], F16,
                                             name=f"q{h}_{tq}",
                                             tag=f"q{h}_{tq}")
            # tq-major creation so the bufs=8 ring pairs tq and tq+2 slots:
            # at(h, tq+2)'s write then waits on oproj(tq)'s reads, which
            # precede it in program order.
            for tq in range(NTQ):
                for h in range(HPC):
                    at_sb[(h, tq)] = pp.tile([128, TQ], BF16,
                                             name=f"a{h}_{tq}", tag="at",
                                             bufs=8)

            # ---------------- Phase P: projections + RoPE -----------------
            with (
                tc.tile_pool(name="pP", bufs=1) as pw,
                tc.tile_pool(name="psP", bufs=1, space="PSUM") as psP,
            ):
                pa = pw
                psA = psP
                wq_sb = pw.tile([128, NCT * TQ], F16, name="wq", tag="wq")
                wk_sb = pw.tile([128, NCT * D], F16, name="wk", tag="wk")
                wv_sb = pw.tile([128, NCT * D], F16, name="wv", tag="wv")
                cs_sb = pw.tile([128, 2 * T], F32, name="cs", tag="cs")
                # weight DMAs: wq in quarters so the first c tiles are ready
                # fast; x half-bundles stream per tq.
                QW = NCT * TQ // 4
                E8 = NCT * TQ // 8
                xtb = [None, None]  # half-bundle ring, bufs=2

                def xq_load(tq, hb):
                    t = pw.tile([128, 16 * TQ], F16, name="xtb", tag="xtb",
                                bufs=2)
                    nc.sync.dma_start(t[:], xq_d[tq * 2 + hb])
                    return t

                # interleave wq eighths with x quarter-slices so the first
                # c tiles stream in at the PE's consumption rate
                xtb0 = pw.tile([128, 16 * TQ], F16, name="xtb", tag="xtb",
                               bufs=2)
                XQ4 = 4 * TQ
                # tiny first chunks (c-tile 0 only) so the PE can start
                # ~5us earlier: queued DMAs share HBM bandwidth round-robin,
                # so the first chunk's latency scales with its size.
                nc.sync.dma_start(wq_sb[:, 0:TQ], wq_d[:, 0:TQ])
                nc.sync.dma_start(xtb0[:, 0:256], xq_d[0, :, 0:256])
                nc.sync.dma_start(xtb0[:, 256:TQ], xq_d[0, :, 256:TQ])
                nc.sync.dma_start(wq_sb[:, TQ:E8], wq_d[:, TQ:E8])
                nc.sync.dma_start(xtb0[:, TQ:XQ4], xq_d[0, :, TQ:XQ4])
                for i in range(1, 4):
                    nc.sync.dma_start(wq_sb[:, i * E8:(i + 1) * E8],
                                      wq_d[:, i * E8:(i + 1) * E8])
                    nc.sync.dma_start(xtb0[:, i * XQ4:(i + 1) * XQ4],
                                      xq_d[0, :, i * XQ4:(i + 1) * XQ4])
                xtb[0] = xtb0
                nc.sync.dma_start(oneb_sb[:], oneb_d[:])
                nc.sync.dma_start(wv_sb[:], wv_d[:])
                nc.sync.dma_start(wk_sb[:], wk_d[:])
                xtb[1] = xq_load(0, 1)
                nc.sync.dma_start(wq_sb[:, 2 * QW:3 * QW],
                                  wq_d[:, 2 * QW:3 * QW])
                nc.sync.dma_start(wq_sb[:, 3 * QW:], wq_d[:, 3 * QW:])
                nc.sync.dma_start(cs_sb[:], cs_d[:])

                def rope(pq, out_tile, tq):
                    # pq: PSUM [128, TQ], rows 0-63 even dims x1, 64-127 odd
                    # dims x2.  out = [x1*c - x2*s ; x2*c + x1*s] via
                    # cc = [c;c], ssn = [s;-s]:
                    #   A = pq * cc ; B = pq * ssn = [x1 s; -x2 s]
                    #   Bsw = swap-halves(B) ; out = A + Bsw
                    cc = cs_sb[:, tq * TQ:(tq + 1) * TQ]
                    ssn = cs_sb[:, T + tq * TQ:T + (tq + 1) * TQ]
                    A = pw.tile([128, TQ], F32, name="ropeA", tag="ropeA",
                                bufs=1)
                    B = pw.tile([128, TQ], F32, name="ropeB", tag="ropeB",
                                bufs=1)
                    Bs = pw.tile([128, TQ], F32, name="ropeS", tag="ropeS",
                                 bufs=2)
                    nc.vector.tensor_mul(A[:], pq[:], cc)
                    nc.vector.tensor_mul(B[:], pq[:], ssn)
                    nc.scalar.copy(Bs[0:64, :], B[64:128, :])
                    nc.scalar.copy(Bs[64:128, :], B[0:64, :])
                    nc.vector.tensor_add(out_tile[:], A[:], Bs[:])

                for tq in range(NTQ):
                    pq = [psP.tile([128, TQ], F32, name="pqk", tag="pqk",
                                   bufs=5) for _ in range(HPC)]
                    pk = psP.tile([128, TQ], F32, name="pqk", tag="pqk",
                                  bufs=5)
                    pvv = psP.tile([128, TQ], F32, name="aux", tag="aux",
                                   bufs=2)
                    deferred = []
                    for c in range(NCT):
                        hb = c // 16
                        cl = c % 16
                        xs = xtb[hb]
                        xcol = cl * TQ
                        st = c == 0
                        sp = c == NCT - 1

                        def kv(c=c, xs=xs, xcol=xcol, st=st, sp=sp):
                            nc.tensor.matmul(
                                pk[:], wk_sb[:, c * D:(c + 1) * D],
                                xs[:, xcol:xcol + TQ], start=st, stop=sp)
                            nc.tensor.matmul(
                                pvv[:], wv_sb[:, c * D:(c + 1) * D],
                                xs[:, xcol:xcol + TQ], start=st, stop=sp,
                                skip_group_check=True)

                        if tq == 0 and c == 0:
                            # N=256 halves: only 64 KB of x needed before the
                            # very first matmul can issue
                            for xh in range(2):
                                for h in range(HPC):
                                    nc.tensor.matmul(
                                        pq[h][:, xh * 256:xh * 256 + 256],
                                        wq_sb[:, c * TQ + h * 128:
                                              c * TQ + h * 128 + 128],
                                        xs[:, xcol + xh * 256:
                                           xcol + xh * 256 + 256],
                                        start=(xh == 0), stop=False)
                        else:
                            for h in range(HPC):
                                nc.tensor.matmul(
                                    pq[h][:],
                                    wq_sb[:, c * TQ + h * 128:c * TQ + h * 128 + 128],
                                    xs[:, xcol:xcol + TQ], start=st, stop=sp)
                        # first pass: run q-only until its weights landed, so
                        # PE isn't queued behind the wk/wv DMAs
                        if tq == 0 and c < 16:
                            deferred.append(kv)
                        else:
                            if deferred:
                                for f in deferred:
                                    f()
                                deferred = []
                            kv()
                        # prefetch the bundle two ahead (slot hb holds bundle
                        # parity hb) as soon as current half's last use is
                        # emitted
                        if cl == 15 and tq * 2 + hb + 2 < 2 * NTQ:
                            nxt = (tq * 2 + hb + 2)
                            xtb[hb] = xq_load(nxt // 2, nxt % 2)
                    if tq == NTQ - 1:
                        # fast bank release: attention tq0's score tiles wait
                        # on these PSUM banks; a single ACT copy frees each
                        # bank ~0.7us after the last matmul instead of the
                        # serial DVE rope muls (~2us per bank).
                        for h in range(HPC):
                            pqs = pw.tile([128, TQ], F32, name="pqs",
                                          tag="pqs", bufs=2)
                            nc.scalar.copy(pqs[:], pq[h][:])
                            rope(pqs, qt_sb[(h, tq)], tq)
                        pks = pw.tile([128, TQ], F32, name="pqs", tag="pqs",
                                      bufs=2)
                        nc.scalar.copy(pks[:], pk[:])
                        rope(pks, kTr[tq], tq)
                    else:
                        for h in range(HPC):
                            rope(pq[h], qt_sb[(h, tq)], tq)
                        rope(pk, kTr[tq], tq)
                    # v: psum [d, t] -> bf16 -> DMA-xbar-transpose -> [t, d]
                    vraw = pw.tile([128, TQ], BF16, name="vraw", tag="vraw",
                                   bufs=2)
                    nc.scalar.copy(vraw[:], pvv[:])
                    nc.sync.dma_start_transpose(
                        v_sb[tq][:].rearrange("p (a b) -> p a b", a=4),
                        vraw[:])

                # ------------- Phase A+O: attention + o_proj ---------------
                wo_sb = pa.tile([128, HPC * 8 * TQ], BF16, name="wo",
                                tag="wo")
                msk_sb = pa.tile([128, len(MASK_ES) * TQ], BF16, name="msk",
                                 tag="msk")
                nc.sync.dma_start(msk_sb[:], msk_d[:])
                HW = HPC * 8 * TQ // 2
                nc.sync.dma_start(wo_sb[:, 0:HW], wo_d[:, 0:HW])
                nc.sync.dma_start(wo_sb[:, HW:], wo_d[:, HW:])

                def attn(h, tq, lookahead=4):
                    q0 = tq * TQ
                    k0s = _k0_list(q0)
                    qt = qt_sb[(h, tq)]
                    sm = psA.tile([128, TQ], F32, name="sm", tag="aux", bufs=2)
                    pv = psA.tile([128, TQ], F32, name="pv", tag="pv", bufs=1)
                    scs = []
                    exs = []

                    def emit_sc(i):
                        k0 = k0s[i]
                        a, b = _trim(q0, k0)
                        sc = psA.tile([128, TQ], F32, name="sc", tag="pqk",
                                      bufs=5)
                        nc.tensor.matmul(
                            sc[:, a:b],
                            kTr[k0 // TQ][:, (k0 % TQ):(k0 % TQ) + 128],
                            qt[:, a:b], start=True, stop=True)
                        scs.append(sc)

                    def emit_ex(i):
                        k0 = k0s[i]
                        a, b = _trim(q0, k0)
                        ex = pa.tile([128, TQ], BF16, name="ex", tag="ex",
                                     bufs=5)
                        nc.scalar.activation(ex[:, a:b], scs[i][:, a:b],
                                             AF.Exp)
                        ms = _mask_span(q0, k0)
                        if ms is not None:
                            ma, mb = max(ms[0], a), min(ms[1], b)
                            if ma < mb:
                                mi = MASK_IDX[k0 - q0]
                                nc.vector.tensor_mul(
                                    ex[:, ma:mb], ex[:, ma:mb],
                                    msk_sb[:, mi * TQ + ma:mi * TQ + mb])
                        exs.append(ex)

                    LOOKAHEAD = lookahead
                    for i in range(min(LOOKAHEAD, len(k0s))):
                        emit_sc(i)
                        emit_ex(i)
                    for i, k0 in enumerate(k0s):
                        a, b = _trim(q0, k0)
                        st = i == 0
                        sp = i == len(k0s) - 1
                        nc.tensor.matmul(sm[:, a:b], oneb_sb[:, :],
                                         exs[i][:, a:b], start=st, stop=sp,
                                         skip_group_check=True)
                        vt = v_sb[k0 // TQ][:, (k0 % TQ) // 128 * D:
                                            (k0 % TQ) // 128 * D + D]
                        nc.tensor.matmul(pv[:, a:b], vt,
                                         exs[i][:, a:b], start=st, stop=sp,
                                         skip_group_check=True)
                        if i + LOOKAHEAD < len(k0s):
                            emit_sc(i + LOOKAHEAD)
                            emit_ex(i + LOOKAHEAD)
                    # sm rows all hold the column sums (ones stationary is
                    # [128, 128]), so the reciprocal is already broadcast;
                    # ~18-bit approx is plenty for softmax denominators.
                    inv = pa.tile([128, TQ], F32, name="inv", tag="inv",
                                  bufs=2)
                    nc.vector.reciprocal_approx_fast(inv[:], sm[:])
                    nc.vector.tensor_mul(at_sb[(h, tq)][:], pv[:], inv[:])

                def oproj(tq):
                    for ts in range(4):
                        t = tq * 4 + ts
                        last_t = tq == NTQ - 1 and ts == 3
                        osb = pa.tile([128, C], BF16, name="osb", tag="osb",
                                      bufs=2)
                        for jg in range(4):
                            ys = [psA.tile([128, TQ], F32, name="ys",
                                           tag="pqk", bufs=5)
                                  for _ in range(2)]
                            for m in range(HPC):
                                a_sl = at_sb[(m, tq)][:, ts * 128:ts * 128 + 128]
                                for u in range(2):
                                    j = jg * 2 + u
                                    nc.tensor.matmul(
                                        ys[u][:], a_sl,
                                        wo_sb[:, (m * 8 + j) * TQ:
                                              (m * 8 + j + 1) * TQ],
                                        start=(m == 0), stop=(m == HPC - 1))
                            # split the psum->sbuf casts across DVE and ACT
                            # so each jg pair finishes sooner
                            nc.vector.tensor_copy(osb[:, (jg * 2) * TQ:
                                                  (jg * 2 + 1) * TQ],
                                                  ys[0][:])
                            nc.scalar.copy(osb[:, (jg * 2 + 1) * TQ:
                                           (jg * 2 + 2) * TQ],
                                           ys[1][:])
                            if last_t:
                                # quarter-stores: shortens the kernel tail
                                nc.sync.dma_start(
                                    y_d[t * 128:(t + 1) * 128,
                                        jg * 1024:(jg + 1) * 1024],
                                    osb[:, jg * 1024:(jg + 1) * 1024])
                            elif jg == 1:
                                nc.sync.dma_start(
                                    y_d[t * 128:(t + 1) * 128, 0:C // 2],
                                    osb[:, 0:C // 2])
                        if not last_t:
                            nc.sync.dma_start(
                                y_d[t * 128:(t + 1) * 128, C // 2:],
                                osb[:, C // 2:])

                # oproj(tq) shifted after attn(h0, tq+1): gives the h3
                # softmax-normalization chain a head of PE work to hide in.
                # First two attn calls use a short lookahead: their sc PSUM
                # slots are freed one-by-one by phase P's trailing ropes, so
                # a deep lookahead would stall the PE on slot availability.
                for tq in range(NTQ):
                    for h in range(HPC):
                        attn(h, tq, lookahead=2 if (tq == 0 and h < 2) else 4)
                        if h == 0 and tq > 0:
                            oproj(tq - 1)
                oproj(NTQ - 1)

    nc.compile()
    return nc


_CACHE = {}


def _get_nc():
    if "nc" not in _CACHE:
        _CACHE["nc"] = _build()
    return _CACHE["nc"]


def _host_prep(x, cos, sin, Wq, Wk, Wv, Wo):
    import ml_dtypes

    BF = ml_dtypes.bfloat16
    inv_sqrt_d = np.float32(1.0 / math.sqrt(D))
    # RoPE row permutation within each head: even dims then odd dims
    perm = np.concatenate(
        [h * D + np.concatenate([np.arange(0, D, 2), np.arange(1, D, 2)])
         for h in range(32)])
    permk = np.concatenate(
        [h * D + np.concatenate([np.arange(0, D, 2), np.arange(1, D, 2)])
         for h in range(8)])
    Wq_p = (Wq[perm] * inv_sqrt_d).astype(np.float32)
    Wk_p = Wk[permk].astype(np.float32)

    # x: (1, T, C) -> (2*NTQ, 128, 16*TQ) bf16 half-bundles
    x0 = np.asarray(x[0], np.float32)
    xq = (x0.reshape(NTQ, TQ, NCT, 128).transpose(0, 2, 3, 1)
          .reshape(NTQ, 2, 16, 128, TQ).transpose(0, 1, 3, 2, 4)
          .reshape(2 * NTQ, 128, 16 * TQ))
    xq = np.ascontiguousarray(xq).astype(np.float16)

    # cs: [[c;c],[s;-s]] (128, 2T) f32
    cosT = cos.T.astype(np.float32)   # (64, T)
    sinT = sin.T.astype(np.float32)
    cs = np.concatenate(
        [np.concatenate([cosT, cosT], 0),
         np.concatenate([sinT, -sinT], 0)], 1)   # (128, 2T)
    cs = np.ascontiguousarray(cs)

    tk = np.arange(128)[:, None]
    tqv = np.arange(TQ)[None, :]
    masks = np.zeros((len(MASK_ES), 128, TQ), np.float32)
    for i, e in enumerate(MASK_ES):
        valid = (tk <= tqv - e) & (tk >= tqv - e - (WINDOW - 1))
        masks[i] = valid.astype(np.float32)
    # (8,128,512) -> (128, 8*512)
    mskt = np.ascontiguousarray(
        masks.transpose(1, 0, 2).reshape(128, len(MASK_ES) * TQ)).astype(BF)

    onesb = np.ones((128, 128), np.float32).astype(BF)

    def tile_w(WT, width):
        # WT: (C, width*? ) column-major weight (C, M) -> (128, NCT*M)
        M = WT.shape[1]
        return np.ascontiguousarray(
            WT.reshape(NCT, 128, M).transpose(1, 0, 2).reshape(128, NCT * M))

    in_maps = []
    for c in range(NCORE):
        wqT = Wq_p[c * MQ:(c + 1) * MQ].T.astype(np.float32)    # (C, 512)
        wkT = Wk_p[c * D:(c + 1) * D].T.astype(np.float32)      # (C, 128)
        wvT = Wv[c * D:(c + 1) * D].T.astype(np.float32)        # (C, 128)
        woT = Wo[:, c * MQ:(c + 1) * MQ].T.astype(np.float32)   # (512, C)
        wo_t = np.ascontiguousarray(
            woT.reshape(HPC, 128, 8, TQ).transpose(1, 0, 2, 3)
            .reshape(128, HPC * 8 * TQ)).astype(BF)
        in_maps.append({
            "xq": xq,
            "wq": tile_w(wqT, TQ).astype(np.float16),
            "wk": tile_w(wkT, D).astype(np.float16),
            "wv": tile_w(wvT, D).astype(np.float16),
            "wo": wo_t,
            "cs": cs,
            "masks": mskt,
            "onesb": onesb,
        })
    return in_maps


def kernel(x, cos, sin, Wq, Wk, Wv, Wo, sliding_window):
    global LAST_EXEC_NS, LAST_RESULTS
    from concourse.bass_utils import run_bass_kernel_spmd

    x = np.asarray(x, dtype=np.float32)
    cos = np.asarray(cos, dtype=np.float32)
    sin = np.asarray(sin, dtype=np.float32)
    Wq = np.asarray(Wq, dtype=np.float32)
    Wk = np.asarray(Wk, dtype=np.float32)
    Wv = np.asarray(Wv, dtype=np.float32)
    Wo = np.asarray(Wo, dtype=np.float32)
    assert int(sliding_window) == WINDOW, sliding_window
    assert x.shape == (1, T, C)

    nc = _get_nc()
    in_maps = _host_prep(x, cos, sin, Wq, Wk, Wv, Wo)

    trace = bool(os.environ.get("KBENCH_TRACE"))
    kw = {}
    if trace:
        kw["trace"] = True
        if os.environ.get("KBENCH_TMPDIR"):
            kw["tmpdir"] = os.environ["KBENCH_TMPDIR"]
    res = run_bass_kernel_spmd(nc, in_maps, list(range(NCORE)), **kw)
    LAST_RESULTS = res
    LAST_EXEC_NS = res.exec_time_ns

    y = np.zeros((T, C), np.float64)
    for r in res.results:
        y += r["y"].astype(np.float64)
    return y.astype(np.float32).reshape(1, T, C)



# revision 16
# speedup vs baseline: 1.0001x; 1.0001x over previous
"""Mixtral sliding-window attention (B=1, T=2048, C=4096, 32 q heads / 8 kv
heads, D=128, window=1024) on 8 TRN2 NeuronCores.

Sharding: tensor-parallel over kv heads — core c owns kv head c and q heads
4c..4c+3.  Each core computes its q/k/v projections, RoPE, sliding-window
attention, and a partial o_proj (its 512 columns of Wo's input dim); the 8
partial (2048, 4096) outputs are summed on the host.

v1 layout strategy:
  - All operands host-pre-tiled into big per-partition-contiguous blocks so
    every HBM load is one large DMA (~35 DMAs total vs ~400): merged weight /
    x / mask loads, one y store per 128-row block.
  - x is bf16 (mixed-dtype matmuls with f32r weights are allowed); q/k path
    stays f32r for precision; v / wo / probs (ex) / attnT are bf16.
  - RoPE: weights row-permuted (even dims -> partitions 0-63, odd -> 64-127);
    per head 3 DVE muls/adds + 2 ACT half copies using a sign-folded sin
    operand [s; -s].
  - v projection emitted directly in natural [t, d] layout (x-tile stationary,
    wv moving) - no PE transposes, no extra copies.
  - scores computed transposed [tk, tq]; softmax denominators via ones-matmul
    accumulated across k tiles; per-tile q-column ranges trimmed to the
    sliding-window support (exact for bf16 ex streams, min-256 for f32r
    score streams).
  - o_proj interleaved after each tq block's attention so PE never waits on
    the softmax DVE tail.
"""
import math
import os
import sys

sys.path.insert(0, "/opt/trn_rl_repo")
import numpy as np

T = 2048
C = 4096
D = 128
NCORE = 8
HPC = 4          # q heads per core
MQ = HPC * D     # 512 q out dims per core
TQ = 512         # tq block
NTQ = T // TQ    # 4
NCT = C // 128   # 32 contraction tiles
WINDOW = 1024
MASK_ES = [0, 128, 256, 384, -640, -768, -896, -1024]
MASK_IDX = {e: i for i, e in enumerate(MASK_ES)}

LAST_EXEC_NS = None
LAST_RESULTS = None


def _k0_list(q0):
    k0_min = max(0, ((q0 - (WINDOW - 1)) // 128) * 128)
    k0_max = ((q0 + TQ - 1) // 128) * 128
    return list(range(k0_min, k0_max + 1, 128))


def _trim(q0, k0):
    """Valid q-column range [a, b) (relative to q0) for k tile [k0, k0+128)."""
    a = max(0, k0 - q0)
    b = min(TQ, k0 + 127 + WINDOW - q0 + 1)
    return a, b


def _mask_span(q0, k0):
    """Column span (relative to q0) needing the partial-validity mask, or
    None.  e >= 0 (causal diagonal): triangle lives in cols [e, e+128).
    e < 0 (window edge): partial cols are (e+1023, e+1151)."""
    e = k0 - q0
    if e >= 0:
        return e, min(TQ, e + 128)
    lo = e + WINDOW  # first col where the window cuts into this tile
    if lo >= TQ:
        return None
    return max(0, lo), min(TQ, e + 127 + WINDOW + 1)


def _build():
    from concourse import bacc, mybir, tile

    F32 = mybir.dt.float32
    F32R = mybir.dt.float32r
    F16 = mybir.dt.float16
    BF16 = mybir.dt.bfloat16
    AF = mybir.ActivationFunctionType

    nc = bacc.Bacc("TRN2", target_bir_lowering=False, debug=False)

    # host-pre-tiled inputs (all per-partition contiguous)
    xq_d = nc.dram_tensor("xq", (2 * NTQ, 128, 16 * TQ), F16,
                          kind="ExternalInput")      # [tq*2+half][p][cl*512+j]
    wq_d = nc.dram_tensor("wq", (128, NCT * TQ), F16,
                          kind="ExternalInput")      # [p][ct*512 + m]
    wk_d = nc.dram_tensor("wk", (128, NCT * D), F16,
                          kind="ExternalInput")      # [p][ct*128 + d]
    wv_d = nc.dram_tensor("wv", (128, NCT * D), F16,
                          kind="ExternalInput")
    wo_d = nc.dram_tensor("wo", (128, HPC * 8 * TQ), BF16,
                          kind="ExternalInput")      # [p][(m*8+j)*512 + col]
    cs_d = nc.dram_tensor("cs", (128, 2 * T), F32,
                          kind="ExternalInput")      # [[c;c], [s;-s]]
    msk_d = nc.dram_tensor("masks", (128, len(MASK_ES) * TQ), BF16,
                           kind="ExternalInput")
    oneb_d = nc.dram_tensor("onesb", (128, 128), BF16, kind="ExternalInput")
    y_d = nc.dram_tensor("y", (T, C), BF16, kind="ExternalOutput")

    with tile.TileContext(nc) as tc:
        with tc.tile_pool(name="persist", bufs=1) as pp:
            oneb_sb = pp.tile([128, 128], BF16, name="oneb", tag="oneb")

            kTr = [pp.tile([128, TQ], F16, name=f"kTr{i}", tag=f"kTr{i}")
                   for i in range(NTQ)]
            # v_sb[tq] holds the tq block's v in natural [t, d] layout as
            # [128, 4*128]: slice [:, tl*128:+128] is t tile tq*4+tl.
            v_sb = [pp.tile([128, 4 * D], BF16, name=f"v{i}", tag=f"v{i}")
                    for i in range(NTQ)]
            qt_sb = {}   # (h, tq) -> f32r [128, TQ] roped q, transposed [d, t]
            at_sb = {}   # (h, tq) -> bf16 [128, TQ] attnT [d, t]
            for h in range(HPC):
                for tq in range(NTQ):
                    qt_sb[(h, tq)] = pp.tile([128, TQ], F16,
                                             name=f"q{h}_{tq}",
                                             tag=f"q{h}_{tq}")
            # tq-major creation so the bufs=8 ring pairs tq and tq+2 slots:
            # at(h, tq+2)'s write then waits on oproj(tq)'s reads, which
            # precede it in program order.
            for tq in range(NTQ):
                for h in range(HPC):
                    at_sb[(h, tq)] = pp.tile([128, TQ], BF16,
                                             name=f"a{h}_{tq}", tag="at",
                                             bufs=8)

            # ---------------- Phase P: projections + RoPE -----------------
            with (
                tc.tile_pool(name="pP", bufs=1) as pw,
                tc.tile_pool(name="psP", bufs=1, space="PSUM") as psP,
            ):
                pa = pw
                psA = psP
                wq_sb = pw.tile([128, NCT * TQ], F16, name="wq", tag="wq")
                wk_sb = pw.tile([128, NCT * D], F16, name="wk", tag="wk")
                wv_sb = pw.tile([128, NCT * D], F16, name="wv", tag="wv")
                cs_sb = pw.tile([128, 2 * T], F32, name="cs", tag="cs")
                # weight DMAs: wq in quarters so the first c tiles are ready
                # fast; x half-bundles stream per tq.
                QW = NCT * TQ // 4
                E8 = NCT * TQ // 8
                xtb = [None, None]  # half-bundle ring, bufs=2

                def xq_load(tq, hb):
                    t = pw.tile([128, 16 * TQ], F16, name="xtb", tag="xtb",
                                bufs=2)
                    nc.sync.dma_start(t[:], xq_d[tq * 2 + hb])
                    return t

                # interleave wq eighths with x quarter-slices so the first
                # c tiles stream in at the PE's consumption rate
                xtb0 = pw.tile([128, 16 * TQ], F16, name="xtb", tag="xtb",
                               bufs=2)
                XQ4 = 4 * TQ
                # tiny first chunks (c-tile 0 only) so the PE can start
                # ~5us earlier: queued DMAs share HBM bandwidth round-robin,
                # so the first chunk's latency scales with its size.
                nc.sync.dma_start(wq_sb[:, 0:TQ], wq_d[:, 0:TQ])
                nc.sync.dma_start(xtb0[:, 0:256], xq_d[0, :, 0:256])
                nc.sync.dma_start(xtb0[:, 256:TQ], xq_d[0, :, 256:TQ])
                nc.sync.dma_start(wq_sb[:, TQ:E8], wq_d[:, TQ:E8])
                nc.sync.dma_start(xtb0[:, TQ:XQ4], xq_d[0, :, TQ:XQ4])
                for i in range(1, 4):
                    nc.sync.dma_start(wq_sb[:, i * E8:(i + 1) * E8],
                                      wq_d[:, i * E8:(i + 1) * E8])
                    nc.sync.dma_start(xtb0[:, i * XQ4:(i + 1) * XQ4],
                                      xq_d[0, :, i * XQ4:(i + 1) * XQ4])
                xtb[0] = xtb0
                nc.sync.dma_start(oneb_sb[:], oneb_d[:])
                nc.sync.dma_start(wv_sb[:], wv_d[:])
                nc.sync.dma_start(wk_sb[:], wk_d[:])
                xtb[1] = xq_load(0, 1)
                nc.sync.dma_start(wq_sb[:, 2 * QW:3 * QW],
                                  wq_d[:, 2 * QW:3 * QW])
                nc.sync.dma_start(wq_sb[:, 3 * QW:], wq_d[:, 3 * QW:])
                nc.sync.dma_start(cs_sb[:], cs_d[:])

                def rope(pq, out_tile, tq):
                    # pq: PSUM [128, TQ], rows 0-63 even dims x1, 64-127 odd
                    # dims x2.  out = [x1*c - x2*s ; x2*c + x1*s] via
                    # cc = [c;c], ssn = [s;-s]:
                    #   A = pq * cc ; B = pq * ssn = [x1 s; -x2 s]
                    #   Bsw = swap-halves(B) ; out = A + Bsw
                    cc = cs_sb[:, tq * TQ:(tq + 1) * TQ]
                    ssn = cs_sb[:, T + tq * TQ:T + (tq + 1) * TQ]
                    A = pw.tile([128, TQ], F32, name="ropeA", tag="ropeA",
                                bufs=1)
                    B = pw.tile([128, TQ], F32, name="ropeB", tag="ropeB",
                                bufs=1)
                    Bs = pw.tile([128, TQ], F32, name="ropeS", tag="ropeS",
                                 bufs=2)
                    nc.vector.tensor_mul(A[:], pq[:], cc)
                    nc.vector.tensor_mul(B[:], pq[:], ssn)
                    nc.scalar.copy(Bs[0:64, :], B[64:128, :])
                    nc.scalar.copy(Bs[64:128, :], B[0:64, :])
                    nc.vector.tensor_add(out_tile[:], A[:], Bs[:])

                for tq in range(NTQ):
                    pq = [psP.tile([128, TQ], F32, name="pqk", tag="pqk",
                                   bufs=5) for _ in range(HPC)]
                    pk = psP.tile([128, TQ], F32, name="pqk", tag="pqk",
                                  bufs=5)
                    pvv = psP.tile([128, TQ], F32, name="aux", tag="aux",
                                   bufs=2)
                    deferred = []
                    for c in range(NCT):
                        hb = c // 16
                        cl = c % 16
                        xs = xtb[hb]
                        xcol = cl * TQ
                        st = c == 0
                        sp = c == NCT - 1

                        def kv(c=c, xs=xs, xcol=xcol, st=st, sp=sp):
                            nc.tensor.matmul(
                                pk[:], wk_sb[:, c * D:(c + 1) * D],
                                xs[:, xcol:xcol + TQ], start=st, stop=sp)
                            nc.tensor.matmul(
                                pvv[:], wv_sb[:, c * D:(c + 1) * D],
                                xs[:, xcol:xcol + TQ], start=st, stop=sp,
                                skip_group_check=True)

                        if tq == 0 and c == 0:
                            # N=256 halves: only 64 KB of x needed before the
                            # very first matmul can issue
                            for xh in range(2):
                                for h in range(HPC):
                                    nc.tensor.matmul(
                                        pq[h][:, xh * 256:xh * 256 + 256],
                                        wq_sb[:, c * TQ + h * 128:
                                              c * TQ + h * 128 + 128],
                                        xs[:, xcol + xh * 256:
                                           xcol + xh * 256 + 256],
                                        start=(xh == 0), stop=False)
                        else:
                            for h in range(HPC):
                                nc.tensor.matmul(
                                    pq[h][:],
                                    wq_sb[:, c * TQ + h * 128:c * TQ + h * 128 + 128],
                                    xs[:, xcol:xcol + TQ], start=st, stop=sp)
                        # first pass: run q-only until its weights landed, so
                        # PE isn't queued behind the wk/wv DMAs
                        if tq == 0 and c < 16:
                            deferred.append(kv)
                        else:
                            if deferred:
                                for f in deferred:
                                    f()
                                deferred = []
                            kv()
                        # prefetch the bundle two ahead (slot hb holds bundle
                        # parity hb) as soon as current half's last use is
                        # emitted
                        if cl == 15 and tq * 2 + hb + 2 < 2 * NTQ:
                            nxt = (tq * 2 + hb + 2)
                            xtb[hb] = xq_load(nxt // 2, nxt % 2)
                    if tq == NTQ - 1:
                        # fast bank release for the first two banks only:
                        # attention tq0's first score tiles wait on them, and
                        # a single ACT copy frees a bank ~0.9us after the last
                        # matmul vs ~2us for the serial DVE rope muls.  The
                        # rest rope straight from PSUM so ACT stays free for
                        # the first exps.
                        for h in range(HPC):
                            if h < 2:
                                pqs = pw.tile([128, TQ], F32, name="pqs",
                                              tag="pqs", bufs=2)
                                nc.scalar.copy(pqs[:], pq[h][:])
                                rope(pqs, qt_sb[(h, tq)], tq)
                            else:
                                rope(pq[h], qt_sb[(h, tq)], tq)
                        rope(pk, kTr[tq], tq)
                    else:
                        for h in range(HPC):
                            rope(pq[h], qt_sb[(h, tq)], tq)
                        rope(pk, kTr[tq], tq)
                    # v: psum [d, t] -> bf16 -> DMA-xbar-transpose -> [t, d]
                    vraw = pw.tile([128, TQ], BF16, name="vraw", tag="vraw",
                                   bufs=2)
                    nc.scalar.copy(vraw[:], pvv[:])
                    nc.sync.dma_start_transpose(
                        v_sb[tq][:].rearrange("p (a b) -> p a b", a=4),
                        vraw[:])

                # ------------- Phase A+O: attention + o_proj ---------------
                wo_sb = pa.tile([128, HPC * 8 * TQ], BF16, name="wo",
                                tag="wo")
                msk_sb = pa.tile([128, len(MASK_ES) * TQ], BF16, name="msk",
                                 tag="msk")
                nc.sync.dma_start(msk_sb[:], msk_d[:])
                HW = HPC * 8 * TQ // 2
                nc.sync.dma_start(wo_sb[:, 0:HW], wo_d[:, 0:HW])
                nc.sync.dma_start(wo_sb[:, HW:], wo_d[:, HW:])

                def attn(h, tq, lookahead=4):
                    q0 = tq * TQ
                    k0s = _k0_list(q0)
                    qt = qt_sb[(h, tq)]
                    sm = psA.tile([128, TQ], F32, name="sm", tag="aux", bufs=2)
                    pv = psA.tile([128, TQ], F32, name="pv", tag="pv", bufs=1)
                    scs = []
                    exs = []

                    def emit_sc(i):
                        k0 = k0s[i]
                        a, b = _trim(q0, k0)
                        sc = psA.tile([128, TQ], F32, name="sc", tag="pqk",
                                      bufs=5)
                        nc.tensor.matmul(
                            sc[:, a:b],
                            kTr[k0 // TQ][:, (k0 % TQ):(k0 % TQ) + 128],
                            qt[:, a:b], start=True, stop=True)
                        scs.append(sc)

                    def emit_ex(i):
                        k0 = k0s[i]
                        a, b = _trim(q0, k0)
                        ex = pa.tile([128, TQ], BF16, name="ex", tag="ex",
                                     bufs=5)
                        nc.scalar.activation(ex[:, a:b], scs[i][:, a:b],
                                             AF.Exp)
                        ms = _mask_span(q0, k0)
                        if ms is not None:
                            ma, mb = max(ms[0], a), min(ms[1], b)
                            if ma < mb:
                                mi = MASK_IDX[k0 - q0]
                                nc.vector.tensor_mul(
                                    ex[:, ma:mb], ex[:, ma:mb],
                                    msk_sb[:, mi * TQ + ma:mi * TQ + mb])
                        exs.append(ex)

                    LOOKAHEAD = lookahead
                    for i in range(min(LOOKAHEAD, len(k0s))):
                        emit_sc(i)
                        emit_ex(i)
                    for i, k0 in enumerate(k0s):
                        a, b = _trim(q0, k0)
                        st = i == 0
                        sp = i == len(k0s) - 1
                        nc.tensor.matmul(sm[:, a:b], oneb_sb[:, :],
                                         exs[i][:, a:b], start=st, stop=sp,
                                         skip_group_check=True)
                        vt = v_sb[k0 // TQ][:, (k0 % TQ) // 128 * D:
                                            (k0 % TQ) // 128 * D + D]
                        nc.tensor.matmul(pv[:, a:b], vt,
                                         exs[i][:, a:b], start=st, stop=sp,
                                         skip_group_check=True)
                        if i + LOOKAHEAD < len(k0s):
                            emit_sc(i + LOOKAHEAD)
                            emit_ex(i + LOOKAHEAD)
                    # sm rows all hold the column sums (ones stationary is
                    # [128, 128]), so the reciprocal is already broadcast;
                    # ~18-bit approx is plenty for softmax denominators.
                    inv = pa.tile([128, TQ], F32, name="inv", tag="inv",
                                  bufs=2)
                    nc.vector.reciprocal_approx_fast(inv[:], sm[:])
                    nc.vector.tensor_mul(at_sb[(h, tq)][:], pv[:], inv[:])

                def oproj(tq):
                    for ts in range(4):
                        t = tq * 4 + ts
                        last_t = tq == NTQ - 1 and ts == 3
                        osb = pa.tile([128, C], BF16, name="osb", tag="osb",
                                      bufs=2)
                        for jg in range(4):
                            ys = [psA.tile([128, TQ], F32, name="ys",
                                           tag="pqk", bufs=5)
                                  for _ in range(2)]
                            for m in range(HPC):
                                a_sl = at_sb[(m, tq)][:, ts * 128:ts * 128 + 128]
                                for u in range(2):
                                    j = jg * 2 + u
                                    nc.tensor.matmul(
                                        ys[u][:], a_sl,
                                        wo_sb[:, (m * 8 + j) * TQ:
                                              (m * 8 + j + 1) * TQ],
                                        start=(m == 0), stop=(m == HPC - 1))
                            # split the psum->sbuf casts across DVE and ACT
                            # so each jg pair finishes sooner
                            nc.vector.tensor_copy(osb[:, (jg * 2) * TQ:
                                                  (jg * 2 + 1) * TQ],
                                                  ys[0][:])
                            nc.scalar.copy(osb[:, (jg * 2 + 1) * TQ:
                                           (jg * 2 + 2) * TQ],
                                           ys[1][:])
                            if last_t:
                                # eighth-stores, one per engine copy, so the
                                # final DMA starts as soon as the DVE half of
                                # the last pair lands
                                nc.sync.dma_start(
                                    y_d[t * 128:(t + 1) * 128,
                                        (jg * 2) * TQ:(jg * 2 + 1) * TQ],
                                    osb[:, (jg * 2) * TQ:(jg * 2 + 1) * TQ])
                                nc.sync.dma_start(
                                    y_d[t * 128:(t + 1) * 128,
                                        (jg * 2 + 1) * TQ:(jg * 2 + 2) * TQ],
                                    osb[:, (jg * 2 + 1) * TQ:
                                        (jg * 2 + 2) * TQ])
                            elif jg == 1:
                                nc.sync.dma_start(
                                    y_d[t * 128:(t + 1) * 128, 0:C // 2],
                                    osb[:, 0:C // 2])
                        if not last_t:
                            nc.sync.dma_start(
                                y_d[t * 128:(t + 1) * 128, C // 2:],
                                osb[:, C // 2:])

                # oproj(tq) shifted after attn(h0, tq+1): gives the h3
                # softmax-normalization chain a head of PE work to hide in.
                # First two attn calls use a short lookahead: their sc PSUM
                # slots are freed one-by-one by phase P's trailing ropes, so
                # a deep lookahead would stall the PE on slot availability.
                for tq in range(NTQ):
                    for h in range(HPC):
                        attn(h, tq, lookahead=2 if (tq == 0 and h < 2) else 4)
                        if h == 0 and tq > 0:
                            oproj(tq - 1)
                oproj(NTQ - 1)

    nc.compile()
    return nc


_CACHE = {}


def _get_nc():
    if "nc" not in _CACHE:
        _CACHE["nc"] = _build()
    return _CACHE["nc"]


def _host_prep(x, cos, sin, Wq, Wk, Wv, Wo):
    import ml_dtypes

    BF = ml_dtypes.bfloat16
    inv_sqrt_d = np.float32(1.0 / math.sqrt(D))
    # RoPE row permutation within each head: even dims then odd dims
    perm = np.concatenate(
        [h * D + np.concatenate([np.arange(0, D, 2), np.arange(1, D, 2)])
         for h in range(32)])
    permk = np.concatenate(
        [h * D + np.concatenate([np.arange(0, D, 2), np.arange(1, D, 2)])
         for h in range(8)])
    Wq_p = (Wq[perm] * inv_sqrt_d).astype(np.float32)
    Wk_p = Wk[permk].astype(np.float32)

    # x: (1, T, C) -> (2*NTQ, 128, 16*TQ) bf16 half-bundles
    x0 = np.asarray(x[0], np.float32)
    xq = (x0.reshape(NTQ, TQ, NCT, 128).transpose(0, 2, 3, 1)
          .reshape(NTQ, 2, 16, 128, TQ).transpose(0, 1, 3, 2, 4)
          .reshape(2 * NTQ, 128, 16 * TQ))
    xq = np.ascontiguousarray(xq).astype(np.float16)

    # cs: [[c;c],[s;-s]] (128, 2T) f32
    cosT = cos.T.astype(np.float32)   # (64, T)
    sinT = sin.T.astype(np.float32)
    cs = np.concatenate(
        [np.concatenate([cosT, cosT], 0),
         np.concatenate([sinT, -sinT], 0)], 1)   # (128, 2T)
    cs = np.ascontiguousarray(cs)

    tk = np.arange(128)[:, None]
    tqv = np.arange(TQ)[None, :]
    masks = np.zeros((len(MASK_ES), 128, TQ), np.float32)
    for i, e in enumerate(MASK_ES):
        valid = (tk <= tqv - e) & (tk >= tqv - e - (WINDOW - 1))
        masks[i] = valid.astype(np.float32)
    # (8,128,512) -> (128, 8*512)
    mskt = np.ascontiguousarray(
        masks.transpose(1, 0, 2).reshape(128, len(MASK_ES) * TQ)).astype(BF)

    onesb = np.ones((128, 128), np.float32).astype(BF)

    def tile_w(WT, width):
        # WT: (C, width*? ) column-major weight (C, M) -> (128, NCT*M)
        M = WT.shape[1]
        return np.ascontiguousarray(
            WT.reshape(NCT, 128, M).transpose(1, 0, 2).reshape(128, NCT * M))

    in_maps = []
    for c in range(NCORE):
        wqT = Wq_p[c * MQ:(c + 1) * MQ].T.astype(np.float32)    # (C, 512)
        wkT = Wk_p[c * D:(c + 1) * D].T.astype(np.float32)      # (C, 128)
        wvT = Wv[c * D:(c + 1) * D].T.astype(np.float32)        # (C, 128)
        woT = Wo[:, c * MQ:(c + 1) * MQ].T.astype(np.float32)   # (512, C)
        wo_t = np.ascontiguousarray(
            woT.reshape(HPC, 128, 8, TQ).transpose(1, 0, 2, 3)
            .reshape(128, HPC * 8 * TQ)).astype(BF)
        in_maps.append({
            "xq": xq,
            "wq": tile_w(wqT, TQ).astype(np.float16),
            "wk": tile_w(wkT, D).astype(np.float16),
            "wv": tile_w(wvT, D).astype(np.float16),
            "wo": wo_t,
            "cs": cs,
            "masks": mskt,
            "onesb": onesb,
        })
    return in_maps


def kernel(x, cos, sin, Wq, Wk, Wv, Wo, sliding_window):
    global LAST_EXEC_NS, LAST_RESULTS
    from concourse.bass_utils import run_bass_kernel_spmd

    x = np.asarray(x, dtype=np.float32)
    cos = np.asarray(cos, dtype=np.float32)
    sin = np.asarray(sin, dtype=np.float32)
    Wq = np.asarray(Wq, dtype=np.float32)
    Wk = np.asarray(Wk, dtype=np.float32)
    Wv = np.asarray(Wv, dtype=np.float32)
    Wo = np.asarray(Wo, dtype=np.float32)
    assert int(sliding_window) == WINDOW, sliding_window
    assert x.shape == (1, T, C)

    nc = _get_nc()
    in_maps = _host_prep(x, cos, sin, Wq, Wk, Wv, Wo)

    trace = bool(os.environ.get("KBENCH_TRACE"))
    kw = {}
    if trace:
        kw["trace"] = True
        if os.environ.get("KBENCH_TMPDIR"):
            kw["tmpdir"] = os.environ["KBENCH_TMPDIR"]
    res = run_bass_kernel_spmd(nc, in_maps, list(range(NCORE)), **kw)
    LAST_RESULTS = res
    LAST_EXEC_NS = res.exec_time_ns

    y = np.zeros((T, C), np.float64)
    for r in res.results:
        y += r["y"].astype(np.float64)
    return y.astype(np.float32).reshape(1, T, C)



# revision 18
# speedup vs baseline: 1.0060x; 1.0059x over previous
"""Mixtral sliding-window attention (B=1, T=2048, C=4096, 32 q heads / 8 kv
heads, D=128, window=1024) on 8 TRN2 NeuronCores.

Sharding: tensor-parallel over kv heads — core c owns kv head c and q heads
4c..4c+3.  Each core computes its q/k/v projections, RoPE, sliding-window
attention, and a partial o_proj (its 512 columns of Wo's input dim); the 8
partial (2048, 4096) outputs are summed on the host.

v1 layout strategy:
  - All operands host-pre-tiled into big per-partition-contiguous blocks so
    every HBM load is one large DMA (~35 DMAs total vs ~400): merged weight /
    x / mask loads, one y store per 128-row block.
  - x is bf16 (mixed-dtype matmuls with f32r weights are allowed); q/k path
    stays f32r for precision; v / wo / probs (ex) / attnT are bf16.
  - RoPE: weights row-permuted (even dims -> partitions 0-63, odd -> 64-127);
    per head 3 DVE muls/adds + 2 ACT half copies using a sign-folded sin
    operand [s; -s].
  - v projection emitted directly in natural [t, d] layout (x-tile stationary,
    wv moving) - no PE transposes, no extra copies.
  - scores computed transposed [tk, tq]; softmax denominators via ones-matmul
    accumulated across k tiles; per-tile q-column ranges trimmed to the
    sliding-window support (exact for bf16 ex streams, min-256 for f32r
    score streams).
  - o_proj interleaved after each tq block's attention so PE never waits on
    the softmax DVE tail.
"""
import math
import os
import sys

sys.path.insert(0, "/opt/trn_rl_repo")
import numpy as np

T = 2048
C = 4096
D = 128
NCORE = 8
HPC = 4          # q heads per core
MQ = HPC * D     # 512 q out dims per core
TQ = 512         # tq block
NTQ = T // TQ    # 4
NCT = C // 128   # 32 contraction tiles
WINDOW = 1024
MASK_ES = [0, 128, 256, 384, -640, -768, -896, -1024]
MASK_IDX = {e: i for i, e in enumerate(MASK_ES)}

LAST_EXEC_NS = None
LAST_RESULTS = None


def _k0_list(q0):
    k0_min = max(0, ((q0 - (WINDOW - 1)) // 128) * 128)
    k0_max = ((q0 + TQ - 1) // 128) * 128
    return list(range(k0_min, k0_max + 1, 128))


def _trim(q0, k0):
    """Valid q-column range [a, b) (relative to q0) for k tile [k0, k0+128)."""
    a = max(0, k0 - q0)
    b = min(TQ, k0 + 127 + WINDOW - q0 + 1)
    return a, b


def _mask_span(q0, k0):
    """Column span (relative to q0) needing the partial-validity mask, or
    None.  e >= 0 (causal diagonal): triangle lives in cols [e, e+128).
    e < 0 (window edge): partial cols are (e+1023, e+1151)."""
    e = k0 - q0
    if e >= 0:
        return e, min(TQ, e + 128)
    lo = e + WINDOW  # first col where the window cuts into this tile
    if lo >= TQ:
        return None
    return max(0, lo), min(TQ, e + 127 + WINDOW + 1)


def _build():
    from concourse import bacc, mybir, tile

    F32 = mybir.dt.float32
    F32R = mybir.dt.float32r
    F16 = mybir.dt.float16
    BF16 = mybir.dt.bfloat16
    AF = mybir.ActivationFunctionType

    nc = bacc.Bacc("TRN2", target_bir_lowering=False, debug=False)

    # host-pre-tiled inputs (all per-partition contiguous)
    xq_d = nc.dram_tensor("xq", (2 * NTQ, 128, 16 * TQ), F16,
                          kind="ExternalInput")      # [tq*2+half][p][cl*512+j]
    wq_d = nc.dram_tensor("wq", (128, NCT * TQ), F16,
                          kind="ExternalInput")      # [p][ct*512 + m]
    wk_d = nc.dram_tensor("wk", (128, NCT * D), F16,
                          kind="ExternalInput")      # [p][ct*128 + d]
    wv_d = nc.dram_tensor("wv", (128, NCT * D), F16,
                          kind="ExternalInput")
    wo_d = nc.dram_tensor("wo", (128, HPC * 8 * TQ), BF16,
                          kind="ExternalInput")      # [p][(m*8+j)*512 + col]
    cs_d = nc.dram_tensor("cs", (128, 2 * T), F32,
                          kind="ExternalInput")      # [[c;c], [s;-s]]
    msk_d = nc.dram_tensor("masks", (128, len(MASK_ES) * TQ), BF16,
                           kind="ExternalInput")
    oneb_d = nc.dram_tensor("onesb", (128, 128), BF16, kind="ExternalInput")
    y_d = nc.dram_tensor("y", (T, C), BF16, kind="ExternalOutput")

    with tile.TileContext(nc) as tc:
        with tc.tile_pool(name="persist", bufs=1) as pp:
            oneb_sb = pp.tile([128, 128], BF16, name="oneb", tag="oneb")

            kTr = [pp.tile([128, TQ], F16, name=f"kTr{i}", tag=f"kTr{i}")
                   for i in range(NTQ)]
            # v_sb[tq] holds the tq block's v in natural [t, d] layout as
            # [128, 4*128]: slice [:, tl*128:+128] is t tile tq*4+tl.
            v_sb = [pp.tile([128, 4 * D], BF16, name=f"v{i}", tag=f"v{i}")
                    for i in range(NTQ)]
            qt_sb = {}   # (h, tq) -> f32r [128, TQ] roped q, transposed [d, t]
            at_sb = {}   # (h, tq) -> bf16 [128, TQ] attnT [d, t]
            for h in range(HPC):
                for tq in range(NTQ):
                    qt_sb[(h, tq)] = pp.tile([128, TQ], F16,
                                             name=f"q{h}_{tq}",
                                             tag=f"q{h}_{tq}")
            # tq-major creation so the bufs=8 ring pairs tq and tq+2 slots:
            # at(h, tq+2)'s write then waits on oproj(tq)'s reads, which
            # precede it in program order.
            for tq in range(NTQ):
                for h in range(HPC):
                    at_sb[(h, tq)] = pp.tile([128, TQ], BF16,
                                             name=f"a{h}_{tq}", tag="at",
                                             bufs=8)

            # ---------------- Phase P: projections + RoPE -----------------
            with (
                tc.tile_pool(name="pP", bufs=1) as pw,
                tc.tile_pool(name="psP", bufs=1, space="PSUM") as psP,
            ):
                pa = pw
                psA = psP
                wq_sb = pw.tile([128, NCT * TQ], F16, name="wq", tag="wq")
                wk_sb = pw.tile([128, NCT * D], F16, name="wk", tag="wk")
                wv_sb = pw.tile([128, NCT * D], F16, name="wv", tag="wv")
                cs_sb = pw.tile([128, 2 * T], F32, name="cs", tag="cs")
                # weight DMAs: wq in quarters so the first c tiles are ready
                # fast; x half-bundles stream per tq.
                QW = NCT * TQ // 4
                E8 = NCT * TQ // 8
                xtb = [None, None]  # half-bundle ring, bufs=2

                def xq_load(tq, hb):
                    t = pw.tile([128, 16 * TQ], F16, name="xtb", tag="xtb",
                                bufs=2)
                    nc.sync.dma_start(t[:], xq_d[tq * 2 + hb])
                    return t

                # interleave wq eighths with x quarter-slices so the first
                # c tiles stream in at the PE's consumption rate
                xtb0 = pw.tile([128, 16 * TQ], F16, name="xtb", tag="xtb",
                               bufs=2)
                XQ4 = 4 * TQ
                # tiny first chunks (c-tile 0 only) so the PE can start
                # ~5us earlier: queued DMAs share HBM bandwidth round-robin,
                # so the first chunk's latency scales with its size.
                nc.sync.dma_start(wq_sb[:, 0:TQ], wq_d[:, 0:TQ])
                nc.sync.dma_start(xtb0[:, 0:TQ], xq_d[0, :, 0:TQ])
                nc.sync.dma_start(wq_sb[:, TQ:E8], wq_d[:, TQ:E8])
                nc.sync.dma_start(xtb0[:, TQ:XQ4], xq_d[0, :, TQ:XQ4])
                for i in range(1, 4):
                    nc.sync.dma_start(wq_sb[:, i * E8:(i + 1) * E8],
                                      wq_d[:, i * E8:(i + 1) * E8])
                    nc.sync.dma_start(xtb0[:, i * XQ4:(i + 1) * XQ4],
                                      xq_d[0, :, i * XQ4:(i + 1) * XQ4])
                xtb[0] = xtb0
                nc.sync.dma_start(oneb_sb[:], oneb_d[:])
                nc.sync.dma_start(wv_sb[:], wv_d[:])
                nc.sync.dma_start(wk_sb[:], wk_d[:])
                xtb[1] = xq_load(0, 1)
                nc.sync.dma_start(wq_sb[:, 2 * QW:3 * QW],
                                  wq_d[:, 2 * QW:3 * QW])
                nc.sync.dma_start(wq_sb[:, 3 * QW:], wq_d[:, 3 * QW:])
                nc.sync.dma_start(cs_sb[:], cs_d[:])

                def rope(pq, out_tile, tq):
                    # pq: PSUM [128, TQ], rows 0-63 even dims x1, 64-127 odd
                    # dims x2.  out = [x1*c - x2*s ; x2*c + x1*s] via
                    # cc = [c;c], ssn = [s;-s]:
                    #   A = pq * cc ; B = pq * ssn = [x1 s; -x2 s]
                    #   Bsw = swap-halves(B) ; out = A + Bsw
                    cc = cs_sb[:, tq * TQ:(tq + 1) * TQ]
                    ssn = cs_sb[:, T + tq * TQ:T + (tq + 1) * TQ]
                    A = pw.tile([128, TQ], F32, name="ropeA", tag="ropeA",
                                bufs=1)
                    B = pw.tile([128, TQ], F32, name="ropeB", tag="ropeB",
                                bufs=1)
                    Bs = pw.tile([128, TQ], F32, name="ropeS", tag="ropeS",
                                 bufs=2)
                    nc.vector.tensor_mul(A[:], pq[:], cc)
                    nc.vector.tensor_mul(B[:], pq[:], ssn)
                    nc.scalar.copy(Bs[0:64, :], B[64:128, :])
                    nc.scalar.copy(Bs[64:128, :], B[0:64, :])
                    nc.vector.tensor_add(out_tile[:], A[:], Bs[:])

                for tq in range(NTQ):
                    pq = [psP.tile([128, TQ], F32, name="pqk", tag="pqk",
                                   bufs=5) for _ in range(HPC)]
                    pk = psP.tile([128, TQ], F32, name="pqk", tag="pqk",
                                  bufs=5)
                    pvv = psP.tile([128, TQ], F32, name="aux", tag="aux",
                                   bufs=2)
                    deferred = []
                    for c in range(NCT):
                        hb = c // 16
                        cl = c % 16
                        xs = xtb[hb]
                        xcol = cl * TQ
                        st = c == 0
                        sp = c == NCT - 1

                        def kv(c=c, xs=xs, xcol=xcol, st=st, sp=sp):
                            nc.tensor.matmul(
                                pk[:], wk_sb[:, c * D:(c + 1) * D],
                                xs[:, xcol:xcol + TQ], start=st, stop=sp)
                            nc.tensor.matmul(
                                pvv[:], wv_sb[:, c * D:(c + 1) * D],
                                xs[:, xcol:xcol + TQ], start=st, stop=sp,
                                skip_group_check=True)

                        for h in range(HPC):
                            nc.tensor.matmul(
                                pq[h][:],
                                wq_sb[:, c * TQ + h * 128:c * TQ + h * 128 + 128],
                                xs[:, xcol:xcol + TQ], start=st, stop=sp)
                        # first pass: run q-only until its weights landed, so
                        # PE isn't queued behind the wk/wv DMAs
                        if tq == 0 and c < 16:
                            deferred.append(kv)
                        else:
                            if deferred:
                                for f in deferred:
                                    f()
                                deferred = []
                            kv()
                        # prefetch the bundle two ahead (slot hb holds bundle
                        # parity hb) as soon as current half's last use is
                        # emitted
                        if cl == 15 and tq * 2 + hb + 2 < 2 * NTQ:
                            nxt = (tq * 2 + hb + 2)
                            xtb[hb] = xq_load(nxt // 2, nxt % 2)
                    if tq == NTQ - 1:
                        # fast bank release for the first two banks only:
                        # attention tq0's first score tiles wait on them, and
                        # a single ACT copy frees a bank ~0.9us after the last
                        # matmul vs ~2us for the serial DVE rope muls.  The
                        # rest rope straight from PSUM so ACT stays free for
                        # the first exps.
                        for h in range(HPC):
                            if h < 2:
                                pqs = pw.tile([128, TQ], F32, name="pqs",
                                              tag="pqs", bufs=2)
                                nc.scalar.copy(pqs[:], pq[h][:])
                                rope(pqs, qt_sb[(h, tq)], tq)
                            else:
                                rope(pq[h], qt_sb[(h, tq)], tq)
                        rope(pk, kTr[tq], tq)
                    else:
                        for h in range(HPC):
                            rope(pq[h], qt_sb[(h, tq)], tq)
                        rope(pk, kTr[tq], tq)
                    # v: psum [d, t] -> bf16 -> DMA-xbar-transpose -> [t, d]
                    vraw = pw.tile([128, TQ], BF16, name="vraw", tag="vraw",
                                   bufs=2)
                    nc.scalar.copy(vraw[:], pvv[:])
                    nc.sync.dma_start_transpose(
                        v_sb[tq][:].rearrange("p (a b) -> p a b", a=4),
                        vraw[:])

                # ------------- Phase A+O: attention + o_proj ---------------
                wo_sb = pa.tile([128, HPC * 8 * TQ], BF16, name="wo",
                                tag="wo")
                msk_sb = pa.tile([128, len(MASK_ES) * TQ], BF16, name="msk",
                                 tag="msk")
                nc.sync.dma_start(msk_sb[:], msk_d[:])
                HW = HPC * 8 * TQ // 2
                nc.sync.dma_start(wo_sb[:, 0:HW], wo_d[:, 0:HW])
                nc.sync.dma_start(wo_sb[:, HW:], wo_d[:, HW:])

                def attn(h, tq, lookahead=4):
                    q0 = tq * TQ
                    k0s = _k0_list(q0)
                    qt = qt_sb[(h, tq)]
                    sm = psA.tile([128, TQ], F32, name="sm", tag="aux", bufs=2)
                    pv = psA.tile([128, TQ], F32, name="pv", tag="pv", bufs=1)
                    scs = []
                    exs = []

                    def emit_sc(i):
                        k0 = k0s[i]
                        a, b = _trim(q0, k0)
                        sc = psA.tile([128, TQ], F32, name="sc", tag="pqk",
                                      bufs=5)
                        nc.tensor.matmul(
                            sc[:, a:b],
                            kTr[k0 // TQ][:, (k0 % TQ):(k0 % TQ) + 128],
                            qt[:, a:b], start=True, stop=True)
                        scs.append(sc)

                    def emit_ex(i):
                        k0 = k0s[i]
                        a, b = _trim(q0, k0)
                        ex = pa.tile([128, TQ], BF16, name="ex", tag="ex",
                                     bufs=5)
                        nc.scalar.activation(ex[:, a:b], scs[i][:, a:b],
                                             AF.Exp)
                        ms = _mask_span(q0, k0)
                        if ms is not None:
                            ma, mb = max(ms[0], a), min(ms[1], b)
                            if ma < mb:
                                mi = MASK_IDX[k0 - q0]
                                nc.vector.tensor_mul(
                                    ex[:, ma:mb], ex[:, ma:mb],
                                    msk_sb[:, mi * TQ + ma:mi * TQ + mb])
                        exs.append(ex)

                    LOOKAHEAD = lookahead
                    for i in range(min(LOOKAHEAD, len(k0s))):
                        emit_sc(i)
                        emit_ex(i)
                    for i, k0 in enumerate(k0s):
                        a, b = _trim(q0, k0)
                        st = i == 0
                        sp = i == len(k0s) - 1
                        nc.tensor.matmul(sm[:, a:b], oneb_sb[:, :],
                                         exs[i][:, a:b], start=st, stop=sp,
                                         skip_group_check=True)
                        vt = v_sb[k0 // TQ][:, (k0 % TQ) // 128 * D:
                                            (k0 % TQ) // 128 * D + D]
                        nc.tensor.matmul(pv[:, a:b], vt,
                                         exs[i][:, a:b], start=st, stop=sp,
                                         skip_group_check=True)
                        if i + LOOKAHEAD < len(k0s):
                            emit_sc(i + LOOKAHEAD)
                            emit_ex(i + LOOKAHEAD)
                    # sm rows all hold the column sums (ones stationary is
                    # [128, 128]), so the reciprocal is already broadcast;
                    # ~18-bit approx is plenty for softmax denominators.
                    inv = pa.tile([128, TQ], F32, name="inv", tag="inv",
                                  bufs=2)
                    nc.vector.reciprocal_approx_fast(inv[:], sm[:])
                    nc.vector.tensor_mul(at_sb[(h, tq)][:], pv[:], inv[:])

                def oproj(tq):
                    for ts in range(4):
                        t = tq * 4 + ts
                        last_t = tq == NTQ - 1 and ts == 3
                        osb = pa.tile([128, C], BF16, name="osb", tag="osb",
                                      bufs=2)
                        for jg in range(4):
                            ys = [psA.tile([128, TQ], F32, name="ys",
                                           tag="pqk", bufs=5)
                                  for _ in range(2)]
                            for m in range(HPC):
                                a_sl = at_sb[(m, tq)][:, ts * 128:ts * 128 + 128]
                                for u in range(2):
                                    j = jg * 2 + u
                                    nc.tensor.matmul(
                                        ys[u][:], a_sl,
                                        wo_sb[:, (m * 8 + j) * TQ:
                                              (m * 8 + j + 1) * TQ],
                                        start=(m == 0), stop=(m == HPC - 1))
                            # split the psum->sbuf casts across DVE and ACT
                            # so each jg pair finishes sooner
                            nc.vector.tensor_copy(osb[:, (jg * 2) * TQ:
                                                  (jg * 2 + 1) * TQ],
                                                  ys[0][:])
                            nc.scalar.copy(osb[:, (jg * 2 + 1) * TQ:
                                           (jg * 2 + 2) * TQ],
                                           ys[1][:])
                            if last_t:
                                # eighth-stores, one per engine copy, so the
                                # final DMA starts as soon as the DVE half of
                                # the last pair lands
                                nc.sync.dma_start(
                                    y_d[t * 128:(t + 1) * 128,
                                        (jg * 2) * TQ:(jg * 2 + 1) * TQ],
                                    osb[:, (jg * 2) * TQ:(jg * 2 + 1) * TQ])
                                nc.sync.dma_start(
                                    y_d[t * 128:(t + 1) * 128,
                                        (jg * 2 + 1) * TQ:(jg * 2 + 2) * TQ],
                                    osb[:, (jg * 2 + 1) * TQ:
                                        (jg * 2 + 2) * TQ])
                            elif jg == 1:
                                nc.sync.dma_start(
                                    y_d[t * 128:(t + 1) * 128, 0:C // 2],
                                    osb[:, 0:C // 2])
                        if not last_t:
                            nc.sync.dma_start(
                                y_d[t * 128:(t + 1) * 128, C // 2:],
                                osb[:, C // 2:])

                # oproj(tq) shifted after attn(h0, tq+1): gives the h3
                # softmax-normalization chain a head of PE work to hide in.
                # First two attn calls use a short lookahead: their sc PSUM
                # slots are freed one-by-one by phase P's trailing ropes, so
                # a deep lookahead would stall the PE on slot availability.
                for tq in range(NTQ):
                    for h in range(HPC):
                        attn(h, tq, lookahead=2 if (tq == 0 and h < 2) else 4)
                        if h == 0 and tq > 0:
                            oproj(tq - 1)
                oproj(NTQ - 1)

    nc.compile()
    return nc


_CACHE = {}


def _get_nc():
    if "nc" not in _CACHE:
        _CACHE["nc"] = _build()
    return _CACHE["nc"]


def _host_prep(x, cos, sin, Wq, Wk, Wv, Wo):
    import ml_dtypes

    BF = ml_dtypes.bfloat16
    inv_sqrt_d = np.float32(1.0 / math.sqrt(D))
    # RoPE row permutation within each head: even dims then odd dims
    perm = np.concatenate(
        [h * D + np.concatenate([np.arange(0, D, 2), np.arange(1, D, 2)])
         for h in range(32)])
    permk = np.concatenate(
        [h * D + np.concatenate([np.arange(0, D, 2), np.arange(1, D, 2)])
         for h in range(8)])
    Wq_p = (Wq[perm] * inv_sqrt_d).astype(np.float32)
    Wk_p = Wk[permk].astype(np.float32)

    # x: (1, T, C) -> (2*NTQ, 128, 16*TQ) bf16 half-bundles
    x0 = np.asarray(x[0], np.float32)
    xq = (x0.reshape(NTQ, TQ, NCT, 128).transpose(0, 2, 3, 1)
          .reshape(NTQ, 2, 16, 128, TQ).transpose(0, 1, 3, 2, 4)
          .reshape(2 * NTQ, 128, 16 * TQ))
    xq = np.ascontiguousarray(xq).astype(np.float16)

    # cs: [[c;c],[s;-s]] (128, 2T) f32
    cosT = cos.T.astype(np.float32)   # (64, T)
    sinT = sin.T.astype(np.float32)
    cs = np.concatenate(
        [np.concatenate([cosT, cosT], 0),
         np.concatenate([sinT, -sinT], 0)], 1)   # (128, 2T)
    cs = np.ascontiguousarray(cs)

    tk = np.arange(128)[:, None]
    tqv = np.arange(TQ)[None, :]
    masks = np.zeros((len(MASK_ES), 128, TQ), np.float32)
    for i, e in enumerate(MASK_ES):
        valid = (tk <= tqv - e) & (tk >= tqv - e - (WINDOW - 1))
        masks[i] = valid.astype(np.float32)
    # (8,128,512) -> (128, 8*512)
    mskt = np.ascontiguousarray(
        masks.transpose(1, 0, 2).reshape(128, len(MASK_ES) * TQ)).astype(BF)

    onesb = np.ones((128, 128), np.float32).astype(BF)

    def tile_w(WT, width):
        # WT: (C, width*? ) column-major weight (C, M) -> (128, NCT*M)
        M = WT.shape[1]
        return np.ascontiguousarray(
            WT.reshape(NCT, 128, M).transpose(1, 0, 2).reshape(128, NCT * M))

    in_maps = []
    for c in range(NCORE):
        wqT = Wq_p[c * MQ:(c + 1) * MQ].T.astype(np.float32)    # (C, 512)
        wkT = Wk_p[c * D:(c + 1) * D].T.astype(np.float32)      # (C, 128)
        wvT = Wv[c * D:(c + 1) * D].T.astype(np.float32)        # (C, 128)
        woT = Wo[:, c * MQ:(c + 1) * MQ].T.astype(np.float32)   # (512, C)
        wo_t = np.ascontiguousarray(
            woT.reshape(HPC, 128, 8, TQ).transpose(1, 0, 2, 3)
            .reshape(128, HPC * 8 * TQ)).astype(BF)
        in_maps.append({
            "xq": xq,
            "wq": tile_w(wqT, TQ).astype(np.float16),
            "wk": tile_w(wkT, D).astype(np.float16),
            "wv": tile_w(wvT, D).astype(np.float16),
            "wo": wo_t,
            "cs": cs,
            "masks": mskt,
            "onesb": onesb,
        })
    return in_maps


def kernel(x, cos, sin, Wq, Wk, Wv, Wo, sliding_window):
    global LAST_EXEC_NS, LAST_RESULTS
    from concourse.bass_utils import run_bass_kernel_spmd

    x = np.asarray(x, dtype=np.float32)
    cos = np.asarray(cos, dtype=np.float32)
    sin = np.asarray(sin, dtype=np.float32)
    Wq = np.asarray(Wq, dtype=np.float32)
    Wk = np.asarray(Wk, dtype=np.float32)
    Wv = np.asarray(Wv, dtype=np.float32)
    Wo = np.asarray(Wo, dtype=np.float32)
    assert int(sliding_window) == WINDOW, sliding_window
    assert x.shape == (1, T, C)

    nc = _get_nc()
    in_maps = _host_prep(x, cos, sin, Wq, Wk, Wv, Wo)

    trace = bool(os.environ.get("KBENCH_TRACE"))
    kw = {}
    if trace:
        kw["trace"] = True
        if os.environ.get("KBENCH_TMPDIR"):
            kw["tmpdir"] = os.environ["KBENCH_TMPDIR"]
    res = run_bass_kernel_spmd(nc, in_maps, list(range(NCORE)), **kw)
    LAST_RESULTS = res
    LAST_EXEC_NS = res.exec_time_ns

    y = np.zeros((T, C), np.float64)
    for r in res.results:
        y += r["y"].astype(np.float64)
    return y.astype(np.float32).reshape(1, T, C)

